# revision 47
# baseline (speedup 1.0000x reference)
"""TRN2 Bass kernel for nn_LogDomainResNet.

The reference network is a signed-log-domain encoding of a plain
real-domain tanh ResNet:

    v0      = sign_x * exp(log_abs_x)
    v_{i+1} = tanh(v_i @ W_i + b_i) + v_i        (7 inner layers)
    t       = v_7 @ W_final
    out     = stack([sign(t), log|t|])

All slog plumbing (per-row max, exp/log per layer) cancels exactly, so the
kernel computes in the real domain. Values stay bounded (|v| < 21), so fp32
range is never an issue.

Precision: the main matmul pass runs in fp32r (the PE rounds both operands
to 11 explicit mantissa bits, verified bit-exact against round-to-nearest-
even on HW), which the TRN2 PE streams at bf16 rate for moving dims >= 256.
The 2^-12 operand-rounding error is recovered with fp8 DoubleRow
corrections that pack K-chunk pairs per instruction (half the per-row cost):

  - v-side (all layers): dv = v - rb11(v), kept as fp8e4m3 at x4096 scale,
    contracted against fp8(W*16): 4 DR matmuls per tile = 1/4 the cost of
    the 8-matmul main pass.
  - W-side (layers in _WCORR): dW8 = fp8((W - rb11(W))*4096) computed
    host-side, contracted against fp8(v) the same way.

The residual stream is maintained as a hi/lo pair: avr (f32r tile; DVE
writes round to the 11-bit grid, which IS the split) and dv8 (fp8; the
sub-grid residual at x4096, doubling as the correction operand). Per
layer: s = dv8/4096 + u; avr' = round(avr + s); dv8' = fp8(4096*((avr -
avr') + s)). The fp8 residual adds ~2^-16-grade noise per layer to the
stream, below the correction's own error floor (CPU-simulated end-to-end).

Engine balance per 512-wide tile: PE 8 fp32r + 4 DR (+4 DR on wcorr
layers); ACT does tanh and the dv8 cast; DVE does s/avr'/d; GpSimd does
(d+s) and the fp8(v) cast. A 6-deep PSUM pool keeps the PE from stalling;
32 warm-up matmuls during the input stage hold the PE p-state at full
clock so real matmuls never pay the ramp.

Layout: activations live transposed ([feature -> partitions, batch ->
free]); the final layer swaps operands (lhsT = v^T tile) to produce t in
natural [batch, feature] layout, so outputs DMA out contiguously.

Sharding: data-parallel over the batch axis, 1024 rows per core x 8 cores.
"""

import numpy as np

_B, _D, _NL = 8192, 1024, 8  # batch, width, layers (7 inner + final)
_NCORES = 8
_BP = _B // _NCORES          # batch rows per core
_P = 128
_KC = _D // _P               # contraction chunks per matmul
_BT = _BP // _P              # batch tiles (input/final stages)
_BCH = 512                   # PSUM free dim
_NT = _D // _P               # out-feature tiles per layer

_WCORR = (1, 3, 5, 7)        # layers with W-side fp8 correction
_SW = 128                    # input strip width
_NS = _BP // _SW             # input strip count

_cached_nc = None
last_results = None  # BassKernelResults from the most recent run (for test.py)


def _build():
    import concourse.mybir as mybir
    from concourse import bacc
    from concourse.tile import TileContext

    f32 = mybir.dt.float32
    f32r = mybir.dt.float32r
    f16 = mybir.dt.float16
    f8 = mybir.dt.float8e4
    AF = mybir.ActivationFunctionType
    ALU = mybir.AluOpType
    DR = mybir.MatmulPerfMode.DoubleRow

    wc_idx = {l: i for i, l in enumerate(_WCORR)}

    nc = bacc.Bacc("TRN2", target_bir_lowering=False, debug=False)
    # inputs arrive strip-major ([strip, partition, chunk, col]), sign as fp8
    d_sgn = nc.dram_tensor("sign_xt", [2, _P, _KC, _BCH], f8, kind="ExternalInput")
    d_lab = nc.dram_tensor("log_abs_xt", [_NS, _P, _KC, _SW], f32r, kind="ExternalInput")
    d_wr = nc.dram_tensor("wr", [_NL, _D, _D], f32r, kind="ExternalInput")
    d_wq = nc.dram_tensor("wq", [_NL, _D, _D], f8, kind="ExternalInput")
    d_dw = nc.dram_tensor("dw", [len(_WCORR), _D, _D], f8, kind="ExternalInput")
    d_bias = nc.dram_tensor("bias", [_P, (_NL - 1) * _NT], f32, kind="ExternalInput")
    d_sgo = nc.dram_tensor("out_sg", [_BP, _D], f8, kind="ExternalOutput")
    d_lgo = nc.dram_tensor("out_lg", [_BP, _D], f16, kind="ExternalOutput")

    with TileContext(nc) as tc:
        with (
            tc.tile_pool(name="const", bufs=1) as constp,
            tc.tile_pool(name="w", bufs=2) as wp,
            tc.tile_pool(name="w8", bufs=2) as w8p,
            tc.tile_pool(name="dw8", bufs=1) as dwp,
            tc.tile_pool(name="avr", bufs=2) as avrp,
            tc.tile_pool(name="dv", bufs=2) as dvp,
            tc.tile_pool(name="v8", bufs=1) as v8p,
            tc.tile_pool(name="tmp", bufs=2) as tmp,
            tc.tile_pool(name="ps", bufs=5, space="PSUM") as ps,
            tc.tile_pool(name="psf", bufs=3, space="PSUM") as psf,
        ):
            # ---- persistent activation state, split per batch half so
            # dependency tracking (tile-granular) lets each half pipeline
            # independently across layers ----
            avrh = [avrp.tile([_P, _KC, _BCH], f32r, tag=f"avr{h}", name=f"avr{h}") for h in range(2)]
            dv8h = [dvp.tile([_P, _KC, _BCH], f8, tag=f"dv8{h}", name=f"dv8{h}") for h in range(2)]
            v8h = [v8p.tile([_P, _KC, _BCH], f8, tag=f"v8{h}", name=f"v8{h}") for h in range(2)]

            # PE warm-up: throwaway matmuls keep the PE p-state at full
            # clock through the input-stage idle gap.
            warm = constp.tile([_P, _BCH], f16, tag="warm")
            nc.gpsimd.memset(warm[:], 0.0)
            half = constp.tile([_P, _BCH], f32, tag="half")
            nc.gpsimd.memset(half[:], 0.5)
            ln4096 = constp.tile([_P, 1], f32, tag="ln4096")
            nc.gpsimd.memset(ln4096[:], 8.317766166719343)
            wps = psf.tile([_P, _BCH], f32, tag="fm")
            for _ in range(76):
                nc.tensor.matmul(
                    wps[:], warm[:, :_P], warm[:], start=True, stop=True,
                )

            # ---- input stage. Inputs and weights share the in-order sync
            # (SP) DMA queue, emission-interleaved so layer-0 weight chunks
            # land between input strips and later layers' weight prefetch
            # cannot jump ahead of input traffic on the (exclusive) DMA
            # engines. Strips 0-7 (the first batch half) are transformed
            # upfront; strips 8-15's transforms are deferred and interleaved
            # into layer-0's first-half tiles so the DVE/ACT/Pool pipelines
            # aren't clogged when layer 0 starts. ----
            wr0_r = d_wr[0].rearrange("(c p) n -> p c n", p=_P)
            wrs0 = wp.tile([_P, _KC, _D], f32r, tag="wr")
            wrs1 = wp.tile([_P, _KC, _D], f32r, tag="wr")
            wr1_r = d_wr[1].rearrange("(c p) n -> p c n", p=_P)

            # raw la strips stage into the avr-new buffers (f32r-typed, DMA
            # preserves bits; unused until layer 0 whose tag-ring allocation
            # then reuses them after the transforms have read everything),
            # and sgn halves stage into the v8 tiles. No ring coupling: all
            # input DMAs fire back-to-back.
            stg = [avrp.tile([_P, _KC, _BCH], f32r, tag=f"avr{h}", name=f"stg{h}")
                   for h in range(2)]

            def transform(t):
                # ist = 4096*v0 = exp(la + ln 4096) * sign; avr = rb11(v0)
                # via round-on-write; dv8 = ist - 4096*avr. Engines alternate
                # by strip parity to balance DVE/Pool.
                h, tq = t // 4, t % 4
                tsl = slice(tq * _SW, (tq + 1) * _SW)
                avr, dv8, v8 = avrh[h], dv8h[h], v8h[h]
                ist = tmp.tile([_P, _KC, _SW], f32, tag="ist", bufs=2)
                nc.scalar.activation(
                    ist[:], stg[h][:, :, tsl].bitcast(f32), AF.Exp,
                    bias=ln4096[:],  # ln(4096): exp(la + ln 4096) = 4096*e^la
                )
                nc.vector.tensor_mul(out=ist[:], in0=ist[:], in1=v8[:, :, tsl])
                eng = nc.gpsimd if t % 2 else nc.vector
                eng.tensor_scalar_mul(
                    out=avr[:, :, tsl], in0=ist[:], scalar1=1.0 / 4096.0
                )
                d4i = tmp.tile([_P, _KC, _SW], f32, tag="ist", bufs=2)
                nc.vector.scalar_tensor_tensor(
                    out=d4i[:], in0=avr[:, :, tsl].bitcast(f32),
                    scalar=-4096.0, in1=ist[:], op0=ALU.mult, op1=ALU.add,
                )
                nc.scalar.activation(dv8[:, :, tsl], d4i[:], AF.Copy)

            nc.sync.dma_start(v8h[0][:], d_sgn[0])
            for t in range(_NS):
                h, tq = t // 4, t % 4
                tsl = slice(tq * _SW, (tq + 1) * _SW)
                nc.sync.dma_start(stg[h][:, :, tsl], d_lab[t])
                if t == 3:
                    nc.sync.dma_start(v8h[1][:], d_sgn[1])
                if t in (1, 2, 3):
                    qsl = slice((t - 1) * 2, t * 2)
                    nc.sync.dma_start(wrs0[:, qsl, :], wr0_r[:, qsl, :])
                if t == 4:
                    qsl = slice(6, 8)
                    nc.sync.dma_start(wrs0[:, qsl, :], wr0_r[:, qsl, :])
                    wqs0 = w8p.tile([_P, _KC, _D], f8, tag="wq")
                    nc.sync.dma_start(
                        wqs0[:], d_wq[0].rearrange("(c p) n -> p c n", p=_P)
                    )
                    dws0 = None
                    if 0 in wc_idx:
                        dws0 = dwp.tile([_P, _KC, _D], f8, tag="dw")
                        nc.sync.dma_start(
                            dws0[:],
                            d_dw[wc_idx[0]].rearrange("(c p) n -> p c n", p=_P),
                        )
                    bias_sb = constp.tile([_P, (_NL - 1) * _NT], f32)
                    nc.sync.dma_start(bias_sb[:], d_bias[:, :])
                transform(t)
            # layer-1 weights follow on the sync queue; their transfers
            # overlap layer-0 compute. Correction weights first: the main
            # wr1 chunks can stream just-in-time under layer 1's c-loop.
            wqs1 = w8p.tile([_P, _KC, _D], f8, tag="wq")
            nc.sync.dma_start(
                wqs1[:], d_wq[1].rearrange("(c p) n -> p c n", p=_P)
            )
            dws1 = None
            if 1 in wc_idx:
                dws1 = dwp.tile([_P, _KC, _D], f8, tag="dw")
                nc.sync.dma_start(
                    dws1[:],
                    d_dw[wc_idx[1]].rearrange("(c p) n -> p c n", p=_P),
                )
            for q in range(4):
                qsl = slice(q * 2, (q + 1) * 2)
                nc.sync.dma_start(wrs1[:, qsl, :], wr1_r[:, qsl, :])

            # ---- 7 inner layers: v = tanh(v @ W + b) + v ----
            for i in range(_NL - 1):
                wcorr = i in wc_idx
                if i == 0:
                    wrs, wqs, dws = wrs0, wqs0, dws0
                else:
                    if i == 1:
                        wrs, wqs, dws = wrs1, wqs1, dws1
                    else:
                        wrs = wp.tile([_P, _KC, _D], f32r, tag="wr")
                        wr_r = d_wr[i].rearrange("(c p) n -> p c n", p=_P)
                        for q in range(4):
                            qsl = slice(q * _KC // 4, (q + 1) * _KC // 4)
                            nc.sync.dma_start(wrs[:, qsl, :], wr_r[:, qsl, :])
                        wqs = w8p.tile([_P, _KC, _D], f8, tag="wq")
                        nc.sync.dma_start(
                            wqs[:], d_wq[i].rearrange("(c p) n -> p c n", p=_P)
                        )
                        if wcorr:
                            dws = dwp.tile([_P, _KC, _D], f8, tag="dw")
                            nc.sync.dma_start(
                                dws[:],
                                d_dw[wc_idx[i]].rearrange("(c p) n -> p c n", p=_P),
                            )
                next_wcorr = (i + 1) in wc_idx
                avr_newh, dv8_newh = [None, None], [None, None]
                for hb in (0, 1):
                    avr, dv8, v8 = avrh[hb], dv8h[hb], v8h[hb]
                    avr_new = avrp.tile([_P, _KC, _BCH], f32r, tag=f"avr{hb}", name=f"avrn{hb}")
                    dv8_new = dvp.tile([_P, _KC, _BCH], f8, tag=f"dv8{hb}", name=f"dv8n{hb}")
                    avr_newh[hb], dv8_newh[hb] = avr_new, dv8_new
                    for n in range(_NT):
                        nsl = slice(n * _P, (n + 1) * _P)
                        # q4096 for this feature chunk, hidden under the MMs
                        q4 = tmp.tile([_P, _BCH], f32, tag="q4", bufs=2)
                        nc.vector.scalar_tensor_tensor(
                            out=q4[:], in0=avr[:, n, :].bitcast(f32),
                            scalar=4096.0, in1=dv8[:, n, :],
                            op0=ALU.mult, op1=ALU.add,
                        )
                        pm = ps.tile([_P, _BCH], f32, tag="mm")
                        for c in range(_KC):
                            nc.tensor.matmul(
                                pm[:], wrs[:, c, nsl], avr[:, c, :],
                                start=(c == 0), stop=False,
                            )
                        ncorr = _KC // 2
                        for c2 in range(ncorr):
                            psl = slice(2 * c2, 2 * c2 + 2)
                            nc.tensor.matmul(
                                pm[:], wqs[:, psl, nsl], dv8[:, psl, :],
                                start=False,
                                stop=(not wcorr and c2 == ncorr - 1),
                                perf_mode=DR,
                            )
                        if wcorr:
                            for c2 in range(ncorr):
                                psl = slice(2 * c2, 2 * c2 + 2)
                                nc.tensor.matmul(
                                    pm[:], dws[:, psl, nsl], v8[:, psl, :],
                                    start=False, stop=(c2 == ncorr - 1),
                                    perf_mode=DR,
                                )
                        # wr is pre-scaled by 2^16 host-side so the fp32r
                        # main pass and both fp8 corrections (4096*16 and
                        # 1*65536 operand scales) share one PSUM group;
                        # tanh then reads PSUM with scale=2^-16.
                        u = tmp.tile([_P, _BCH], f32, tag="u", bufs=2)
                        nc.scalar.activation(
                            u[:], pm[:], AF.Tanh, scale=2.0 ** -16,
                            bias=bias_sb[:, i * _NT + n : i * _NT + n + 1],
                        )
                        # All in x4096 scale (exact powers of two):
                        # q4096 = 4096*v = 4096*avr + dv8 (pre-computable,
                        # overlaps this tile's matmuls); qu4096 = 4096*u +
                        # q4096; avr' = rb11(qu4096/4096) via Pool
                        # round-on-write; dv8' = qu4096 - 4096*avr'.
                        qu4 = tmp.tile([_P, _BCH], f32, tag="qu4", bufs=2)
                        nc.vector.scalar_tensor_tensor(
                            out=qu4[:], in0=u[:], scalar=4096.0,
                            in1=q4[:], op0=ALU.mult, op1=ALU.add,
                        )
                        nc.gpsimd.tensor_scalar_mul(
                            out=avr_new[:, n, :], in0=qu4[:],
                            scalar1=1.0 / 4096.0,
                        )
                        d4 = tmp.tile([_P, _BCH], f32, tag="d4", bufs=2)
                        nc.vector.scalar_tensor_tensor(
                            out=d4[:],
                            in0=avr_new[:, n, :].bitcast(f32),
                            scalar=-4096.0, in1=qu4[:],
                            op0=ALU.mult, op1=ALU.add,
                        )
                        nc.scalar.activation(
                            dv8_new[:, n, :], d4[:], AF.Copy
                        )
                        if next_wcorr:
                            nc.scalar.activation(
                                v8[:, n, :],
                                avr_new[:, n, :].bitcast(f32), AF.Copy,
                            )
                avrh, dv8h = avr_newh, dv8_newh

            # ---- final layer: t = v @ W_f, out = [sign(t), log|t|] ----
            wrf = wp.tile([_P, _KC, _D], f32r, tag="wr")
            nc.sync.dma_start(
                wrf[:], d_wr[_NL - 1].rearrange("(c p) n -> p c n", p=_P)
            )
            wqf = w8p.tile([_P, _KC, _D], f8, tag="wq")
            nc.sync.dma_start(
                wqf[:], d_wq[_NL - 1].rearrange("(c p) n -> p c n", p=_P)
            )
            fcorr = (_NL - 1) in wc_idx
            if fcorr:
                dwf = dwp.tile([_P, _KC, _D], f8, tag="dw")
                nc.sync.dma_start(
                    dwf[:],
                    d_dw[wc_idx[_NL - 1]].rearrange("(c p) n -> p c n", p=_P),
                )
            for bt in range(_BT):
                h, btq = bt // 4, bt % 4
                avr, dv8, v8 = avrh[h], dv8h[h], v8h[h]
                bsl = slice(btq * _P, (btq + 1) * _P)
                for j0 in (0, _BCH):
                    nsl = slice(j0, j0 + _BCH)
                    pm = psf.tile([_P, _BCH], f32, tag="fm")
                    for c in range(_KC):
                        nc.tensor.matmul(
                            pm[:], avr[:, c, bsl], wrf[:, c, nsl],
                            start=(c == 0), stop=False,
                        )
                    ncorr = _KC // 2
                    for c2 in range(ncorr):
                        psl = slice(2 * c2, 2 * c2 + 2)
                        nc.tensor.matmul(
                            pm[:], dv8[:, psl, bsl], wqf[:, psl, nsl],
                            start=False,
                            stop=(not fcorr and c2 == ncorr - 1),
                            perf_mode=DR,
                        )
                    if fcorr:
                        for c2 in range(ncorr):
                            psl = slice(2 * c2, 2 * c2 + 2)
                            nc.tensor.matmul(
                                pm[:], v8[:, psl, bsl], dwf[:, psl, nsl],
                                start=False, stop=(c2 == ncorr - 1),
                                perf_mode=DR,
                            )
                    # sign is invariant to the positive 2^16 scale, so both
                    # outputs read PSUM directly; no separate combine needed.
                    # sign on DVE: (pm >= 0) - 0.5 -> {-0.5, +0.5} in fp8;
                    # the host maps back to +-1
                    sg = tmp.tile([_P, _BCH], f8, tag="fsg", bufs=2)
                    nc.vector.scalar_tensor_tensor(
                        out=sg[:], in0=pm[:], scalar=0.0, in1=half[:],
                        op0=ALU.is_ge, op1=ALU.subtract,
                    )
                    ab = tmp.tile([_P, _BCH], f32, tag="u", bufs=2)
                    nc.scalar.activation(ab[:], pm[:], AF.Abs, scale=2.0 ** -16)
                    lg = tmp.tile([_P, _BCH], f16, tag="lg", bufs=2)
                    nc.scalar.activation(lg[:], ab[:], AF.Ln)
                    # outputs split across the ACT and Pool DMA queues
                    # (both near-idle at this stage), off the weight queue
                    gsl = slice(bt * _P, (bt + 1) * _P)
                    nc.scalar.dma_start(d_sgo[gsl, nsl], sg[:])
                    nc.sync.dma_start(d_lgo[gsl, nsl], lg[:])
    nc.compile()
    return nc


def _rb11(x):
    """Round to 11 explicit mantissa bits (round-to-nearest-even), matching
    the TRN2 fp32r operand rounding bit-for-bit."""
    x = np.asarray(x, dtype=np.float64)
    with np.errstate(divide="ignore", invalid="ignore"):
        e = np.floor(np.log2(np.abs(x)))
    e = np.where(np.isfinite(e), e, 0.0)
    s = 2.0 ** (e - 11)
    return np.round(x / s) * s


def kernel(sign_x, log_abs_x, inner_kernels, final_kernel):
    global _cached_nc, last_results
    import ml_dtypes
    from concourse.bass_utils import run_bass_kernel_spmd

    if _cached_nc is None:
        _cached_nc = _build()
    nc = _cached_nc

    f8 = ml_dtypes.float8_e4m3
    # strip/half-major packing [chunk-of-batch, partition, chunk, col] so
    # each DMA reads contiguous >=512B runs per partition
    def _pack(x, w):  # x: [B, D] -> [B//w, P, KC, w]
        xt = np.asarray(x).T  # [D, B]
        return np.ascontiguousarray(
            xt.reshape(_KC, _P, _B // w, w).transpose(2, 1, 0, 3)
        )

    sign_p = _pack(np.asarray(sign_x, dtype=np.float32), _BCH).astype(f8)
    lab_p = _pack(np.asarray(log_abs_x, dtype=np.float32), _SW)
    ns_core = _NS  # strips per core
    ik = np.asarray(inner_kernels, dtype=np.float32)
    fk = np.asarray(final_kernel, dtype=np.float32)

    W = np.concatenate([ik[:, :_D, :], fk[None]], axis=0)  # [8, 1024, 1024]
    # One PSUM accumulation group at scale 2^16: the fp32r main weights are
    # pre-scaled by 2^16 (exact power of two, commutes with the PE's 11-bit
    # rounding); the corrections' operand scales multiply out to 2^16 too
    # (dv8 at 4096 x wq8 at 16; v8 at 1 x dw8 at 65536). tanh/abs read PSUM
    # with scale=2^-16.
    Wr = np.ascontiguousarray((W.astype(np.float64) * 65536.0).astype(np.float32))
    Wq = (W.astype(np.float64) * 16.0).astype(f8)
    dW = ((W.astype(np.float64) - _rb11(W)) * 65536.0).astype(f8)
    dW = np.ascontiguousarray(dW[list(_WCORR)] if _WCORR else dW[:1] * 0)
    bias = np.ascontiguousarray(
        ik[:, _D, :].reshape(_NL - 1, _NT, _P).transpose(2, 0, 1).reshape(_P, -1)
    )  # [128, 56]: column (l*8+t) holds bias[l, t*128+p] on partition p

    in_maps = []
    for cid in range(_NCORES):
        sl = slice(cid * ns_core, (cid + 1) * ns_core)
        in_maps.append({
            "sign_xt": np.ascontiguousarray(sign_p[2 * cid : 2 * cid + 2]),
            "log_abs_xt": np.ascontiguousarray(lab_p[sl]),
            "wr": Wr,
            "wq": np.ascontiguousarray(Wq),
            "dw": dW,
            "bias": bias,
        })

    last_results = run_bass_kernel_spmd(nc, in_maps, core_ids=list(range(_NCORES)))
    sg = np.sign(np.concatenate(
        [r["out_sg"].astype(np.float32) for r in last_results.results], axis=0
    ))
    lg = np.concatenate(
        [r["out_lg"].astype(np.float32) for r in last_results.results], axis=0
    )
    return np.stack([sg, lg], axis=0)


# revision 48
# speedup vs baseline: 1.0431x; 1.0431x over previous
"""TRN2 Bass kernel for nn_LogDomainResNet.

The reference network is a signed-log-domain encoding of a plain
real-domain tanh ResNet:

    v0      = sign_x * exp(log_abs_x)
    v_{i+1} = tanh(v_i @ W_i + b_i) + v_i        (7 inner layers)
    t       = v_7 @ W_final
    out     = stack([sign(t), log|t|])

All slog plumbing (per-row max, exp/log per layer) cancels exactly, so the
kernel computes in the real domain. Values stay bounded (|v| < 21), so fp32
range is never an issue.

Precision: the main matmul pass runs in fp32r (the PE rounds both operands
to 11 explicit mantissa bits, verified bit-exact against round-to-nearest-
even on HW), which the TRN2 PE streams at bf16 rate for moving dims >= 256.
The 2^-12 operand-rounding error is recovered with fp8 DoubleRow
corrections that pack K-chunk pairs per instruction (half the per-row cost):

  - v-side (all layers): dv = v - rb11(v), kept as fp8e4m3 at x4096 scale,
    contracted against fp8(W*16): 4 DR matmuls per tile = 1/4 the cost of
    the 8-matmul main pass.
  - W-side (layers in _WCORR): dW8 = fp8((W - rb11(W))*4096) computed
    host-side, contracted against fp8(v) the same way.

The residual stream is maintained as a hi/lo pair: avr (f32r tile; DVE
writes round to the 11-bit grid, which IS the split) and dv8 (fp8; the
sub-grid residual at x4096, doubling as the correction operand). Per
layer: s = dv8/4096 + u; avr' = round(avr + s); dv8' = fp8(4096*((avr -
avr') + s)). The fp8 residual adds ~2^-16-grade noise per layer to the
stream, below the correction's own error floor (CPU-simulated end-to-end).

Engine balance per 512-wide tile: PE 8 fp32r + 4 DR (+4 DR on wcorr
layers); ACT does tanh and the dv8 cast; DVE does s/avr'/d; GpSimd does
(d+s) and the fp8(v) cast. A 6-deep PSUM pool keeps the PE from stalling;
32 warm-up matmuls during the input stage hold the PE p-state at full
clock so real matmuls never pay the ramp.

Layout: activations live transposed ([feature -> partitions, batch ->
free]); the final layer swaps operands (lhsT = v^T tile) to produce t in
natural [batch, feature] layout, so outputs DMA out contiguously.

Sharding: data-parallel over the batch axis, 1024 rows per core x 8 cores.
"""

import numpy as np

_B, _D, _NL = 8192, 1024, 8  # batch, width, layers (7 inner + final)
_NCORES = 8
_BP = _B // _NCORES          # batch rows per core
_P = 128
_KC = _D // _P               # contraction chunks per matmul
_BT = _BP // _P              # batch tiles (input/final stages)
_BCH = 512                   # PSUM free dim
_NT = _D // _P               # out-feature tiles per layer

_WCORR = (1, 3, 5, 7)        # layers with W-side fp8 correction
_SW = 128                    # input strip width
_NS = _BP // _SW             # input strip count

_cached_nc = None
last_results = None  # BassKernelResults from the most recent run (for test.py)


def _build():
    import concourse.mybir as mybir
    from concourse import bacc
    from concourse.tile import TileContext

    f32 = mybir.dt.float32
    f32r = mybir.dt.float32r
    f16 = mybir.dt.float16
    f8 = mybir.dt.float8e4
    AF = mybir.ActivationFunctionType
    ALU = mybir.AluOpType
    DR = mybir.MatmulPerfMode.DoubleRow

    wc_idx = {l: i for i, l in enumerate(_WCORR)}

    nc = bacc.Bacc("TRN2", target_bir_lowering=False, debug=False)
    # inputs arrive strip-major ([strip, partition, chunk, col]), sign as fp8
    d_sgn = nc.dram_tensor("sign_xt", [2, _P, _KC, _BCH], f8, kind="ExternalInput")
    d_lab = nc.dram_tensor("log_abs_xt", [_NS, _P, _KC, _SW], f32r, kind="ExternalInput")
    d_wr = nc.dram_tensor("wr", [_NL, _D, _D], f32r, kind="ExternalInput")
    d_wq = nc.dram_tensor("wq", [_NL, _D, _D], f8, kind="ExternalInput")
    d_dw = nc.dram_tensor("dw", [len(_WCORR), _D, _D], f8, kind="ExternalInput")
    d_bias = nc.dram_tensor("bias", [_P, (_NL - 1) * _NT], f32, kind="ExternalInput")
    d_sgo = nc.dram_tensor("out_sg", [_BP, _D], f8, kind="ExternalOutput")
    d_lgo = nc.dram_tensor("out_lg", [_BP, _D], f16, kind="ExternalOutput")

    with TileContext(nc) as tc:
        with (
            tc.tile_pool(name="const", bufs=1) as constp,
            tc.tile_pool(name="w", bufs=2) as wp,
            tc.tile_pool(name="w8", bufs=2) as w8p,
            tc.tile_pool(name="dw8", bufs=1) as dwp,
            tc.tile_pool(name="avr", bufs=2) as avrp,
            tc.tile_pool(name="dv", bufs=2) as dvp,
            tc.tile_pool(name="v8", bufs=1) as v8p,
            tc.tile_pool(name="tmp", bufs=2) as tmp,
            tc.tile_pool(name="ps", bufs=5, space="PSUM") as ps,
            tc.tile_pool(name="psf", bufs=3, space="PSUM") as psf,
        ):
            # ---- persistent activation state, split per batch half so
            # dependency tracking (tile-granular) lets each half pipeline
            # independently across layers ----
            avrh = [avrp.tile([_P, _KC, _BCH], f32r, tag=f"avr{h}", name=f"avr{h}") for h in range(2)]
            dv8h = [dvp.tile([_P, _KC, _BCH], f8, tag=f"dv8{h}", name=f"dv8{h}") for h in range(2)]
            v8h = [v8p.tile([_P, _KC, _BCH], f8, tag=f"v8{h}", name=f"v8{h}") for h in range(2)]

            # PE warm-up: throwaway matmuls keep the PE p-state at full
            # clock through the input-stage idle gap.
            warm = constp.tile([_P, _BCH], f16, tag="warm")
            nc.gpsimd.memset(warm[:], 0.0)
            half = constp.tile([_P, _BCH], f32, tag="half")
            nc.gpsimd.memset(half[:], 0.5)
            ln4096 = constp.tile([_P, 1], f32, tag="ln4096")
            nc.gpsimd.memset(ln4096[:], 8.317766166719343)
            wps = psf.tile([_P, _BCH], f32, tag="fm")
            for _ in range(76):
                nc.tensor.matmul(
                    wps[:], warm[:, :_P], warm[:], start=True, stop=True,
                )

            # ---- input stage. Inputs and weights share the in-order sync
            # (SP) DMA queue, emission-interleaved so layer-0 weight chunks
            # land between input strips and later layers' weight prefetch
            # cannot jump ahead of input traffic on the (exclusive) DMA
            # engines. Strips 0-7 (the first batch half) are transformed
            # upfront; strips 8-15's transforms are deferred and interleaved
            # into layer-0's first-half tiles so the DVE/ACT/Pool pipelines
            # aren't clogged when layer 0 starts. ----
            wr0_r = d_wr[0].rearrange("(c p) n -> p c n", p=_P)
            wrs0 = wp.tile([_P, _KC, _D], f32r, tag="wr")
            wrs1 = wp.tile([_P, _KC, _D], f32r, tag="wr")
            wr1_r = d_wr[1].rearrange("(c p) n -> p c n", p=_P)

            # raw la strips stage into the avr-new buffers (f32r-typed, DMA
            # preserves bits; unused until layer 0 whose tag-ring allocation
            # then reuses them after the transforms have read everything),
            # and sgn halves stage into the v8 tiles. No ring coupling: all
            # input DMAs fire back-to-back.
            stg = [avrp.tile([_P, _KC, _BCH], f32r, tag=f"avr{h}", name=f"stg{h}")
                   for h in range(2)]

            def transform(t):
                # ist = 4096*v0 = exp(la + ln 4096) * sign; avr = rb11(v0)
                # via round-on-write; dv8 = ist - 4096*avr. Engines alternate
                # by strip parity to balance DVE/Pool.
                h, tq = t // 4, t % 4
                tsl = slice(tq * _SW, (tq + 1) * _SW)
                avr, dv8, v8 = avrh[h], dv8h[h], v8h[h]
                ist = tmp.tile([_P, _KC, _SW], f32, tag="ist", bufs=2)
                nc.scalar.activation(
                    ist[:], stg[h][:, :, tsl].bitcast(f32), AF.Exp,
                    bias=ln4096[:],  # ln(4096): exp(la + ln 4096) = 4096*e^la
                )
                nc.vector.tensor_mul(out=ist[:], in0=ist[:], in1=v8[:, :, tsl])
                eng = nc.gpsimd if t % 2 else nc.vector
                eng.tensor_scalar_mul(
                    out=avr[:, :, tsl], in0=ist[:], scalar1=1.0 / 4096.0
                )
                nc.vector.scalar_tensor_tensor(
                    out=dv8[:, :, tsl], in0=avr[:, :, tsl].bitcast(f32),
                    scalar=-4096.0, in1=ist[:], op0=ALU.mult, op1=ALU.add,
                )

            nc.sync.dma_start(v8h[0][:], d_sgn[0])
            for t in range(_NS):
                h, tq = t // 4, t % 4
                tsl = slice(tq * _SW, (tq + 1) * _SW)
                nc.sync.dma_start(stg[h][:, :, tsl], d_lab[t])
                if t == 3:
                    nc.sync.dma_start(v8h[1][:], d_sgn[1])
                if t in (1, 2, 3):
                    qsl = slice((t - 1) * 2, t * 2)
                    nc.sync.dma_start(wrs0[:, qsl, :], wr0_r[:, qsl, :])
                if t == 4:
                    qsl = slice(6, 8)
                    nc.sync.dma_start(wrs0[:, qsl, :], wr0_r[:, qsl, :])
                    wqs0 = w8p.tile([_P, _KC, _D], f8, tag="wq")
                    nc.sync.dma_start(
                        wqs0[:], d_wq[0].rearrange("(c p) n -> p c n", p=_P)
                    )
                    dws0 = None
                    if 0 in wc_idx:
                        dws0 = dwp.tile([_P, _KC, _D], f8, tag="dw")
                        nc.sync.dma_start(
                            dws0[:],
                            d_dw[wc_idx[0]].rearrange("(c p) n -> p c n", p=_P),
                        )
                    bias_sb = constp.tile([_P, (_NL - 1) * _NT], f32)
                    nc.sync.dma_start(bias_sb[:], d_bias[:, :])
                transform(t)
            # layer-1 weights follow on the sync queue; their transfers
            # overlap layer-0 compute. Correction weights first: the main
            # wr1 chunks can stream just-in-time under layer 1's c-loop.
            wqs1 = w8p.tile([_P, _KC, _D], f8, tag="wq")
            nc.sync.dma_start(
                wqs1[:], d_wq[1].rearrange("(c p) n -> p c n", p=_P)
            )
            dws1 = None
            if 1 in wc_idx:
                dws1 = dwp.tile([_P, _KC, _D], f8, tag="dw")
                nc.sync.dma_start(
                    dws1[:],
                    d_dw[wc_idx[1]].rearrange("(c p) n -> p c n", p=_P),
                )
            for q in range(4):
                qsl = slice(q * 2, (q + 1) * 2)
                nc.sync.dma_start(wrs1[:, qsl, :], wr1_r[:, qsl, :])

            # ---- 7 inner layers: v = tanh(v @ W + b) + v ----
            for i in range(_NL - 1):
                wcorr = i in wc_idx
                if i == 0:
                    wrs, wqs, dws = wrs0, wqs0, dws0
                else:
                    if i == 1:
                        wrs, wqs, dws = wrs1, wqs1, dws1
                    else:
                        wrs = wp.tile([_P, _KC, _D], f32r, tag="wr")
                        wr_r = d_wr[i].rearrange("(c p) n -> p c n", p=_P)
                        for q in range(4):
                            qsl = slice(q * _KC // 4, (q + 1) * _KC // 4)
                            nc.sync.dma_start(wrs[:, qsl, :], wr_r[:, qsl, :])
                        wqs = w8p.tile([_P, _KC, _D], f8, tag="wq")
                        nc.sync.dma_start(
                            wqs[:], d_wq[i].rearrange("(c p) n -> p c n", p=_P)
                        )
                        if wcorr:
                            dws = dwp.tile([_P, _KC, _D], f8, tag="dw")
                            nc.sync.dma_start(
                                dws[:],
                                d_dw[wc_idx[i]].rearrange("(c p) n -> p c n", p=_P),
                            )
                next_wcorr = (i + 1) in wc_idx
                avr_newh, dv8_newh = [None, None], [None, None]
                for hb in (0, 1):
                    avr, dv8, v8 = avrh[hb], dv8h[hb], v8h[hb]
                    avr_new = avrp.tile([_P, _KC, _BCH], f32r, tag=f"avr{hb}", name=f"avrn{hb}")
                    dv8_new = dvp.tile([_P, _KC, _BCH], f8, tag=f"dv8{hb}", name=f"dv8n{hb}")
                    avr_newh[hb], dv8_newh[hb] = avr_new, dv8_new
                    for n in range(_NT):
                        nsl = slice(n * _P, (n + 1) * _P)
                        # q4096 for this feature chunk, hidden under the MMs
                        q4 = tmp.tile([_P, _BCH], f32, tag="q4", bufs=2)
                        nc.vector.scalar_tensor_tensor(
                            out=q4[:], in0=avr[:, n, :].bitcast(f32),
                            scalar=4096.0, in1=dv8[:, n, :],
                            op0=ALU.mult, op1=ALU.add,
                        )
                        pm = ps.tile([_P, _BCH], f32, tag="mm")
                        for c in range(_KC):
                            nc.tensor.matmul(
                                pm[:], wrs[:, c, nsl], avr[:, c, :],
                                start=(c == 0), stop=False,
                            )
                        ncorr = _KC // 2
                        for c2 in range(ncorr):
                            psl = slice(2 * c2, 2 * c2 + 2)
                            nc.tensor.matmul(
                                pm[:], wqs[:, psl, nsl], dv8[:, psl, :],
                                start=False,
                                stop=(not wcorr and c2 == ncorr - 1),
                                perf_mode=DR,
                            )
                        if wcorr:
                            for c2 in range(ncorr):
                                psl = slice(2 * c2, 2 * c2 + 2)
                                nc.tensor.matmul(
                                    pm[:], dws[:, psl, nsl], v8[:, psl, :],
                                    start=False, stop=(c2 == ncorr - 1),
                                    perf_mode=DR,
                                )
                        # wr is pre-scaled by 2^16 host-side so the fp32r
                        # main pass and both fp8 corrections (4096*16 and
                        # 1*65536 operand scales) share one PSUM group;
                        # tanh then reads PSUM with scale=2^-16.
                        u = tmp.tile([_P, _BCH], f32, tag="u", bufs=2)
                        nc.scalar.activation(
                            u[:], pm[:], AF.Tanh, scale=2.0 ** -16,
                            bias=bias_sb[:, i * _NT + n : i * _NT + n + 1],
                        )
                        # All in x4096 scale (exact powers of two):
                        # q4096 = 4096*v = 4096*avr + dv8 (pre-computable,
                        # overlaps this tile's matmuls); qu4096 = 4096*u +
                        # q4096; avr' = rb11(qu4096/4096) via Pool
                        # round-on-write; dv8' = qu4096 - 4096*avr'.
                        qu4 = tmp.tile([_P, _BCH], f32, tag="qu4", bufs=2)
                        nc.vector.scalar_tensor_tensor(
                            out=qu4[:], in0=u[:], scalar=4096.0,
                            in1=q4[:], op0=ALU.mult, op1=ALU.add,
                        )
                        nc.gpsimd.tensor_scalar_mul(
                            out=avr_new[:, n, :], in0=qu4[:],
                            scalar1=1.0 / 4096.0,
                        )
                        nc.vector.scalar_tensor_tensor(
                            out=dv8_new[:, n, :],
                            in0=avr_new[:, n, :].bitcast(f32),
                            scalar=-4096.0, in1=qu4[:],
                            op0=ALU.mult, op1=ALU.add,
                        )
                        if next_wcorr:
                            nc.scalar.activation(
                                v8[:, n, :],
                                avr_new[:, n, :].bitcast(f32), AF.Copy,
                            )
                avrh, dv8h = avr_newh, dv8_newh

            # ---- final layer: t = v @ W_f, out = [sign(t), log|t|] ----
            wrf = wp.tile([_P, _KC, _D], f32r, tag="wr")
            nc.sync.dma_start(
                wrf[:], d_wr[_NL - 1].rearrange("(c p) n -> p c n", p=_P)
            )
            wqf = w8p.tile([_P, _KC, _D], f8, tag="wq")
            nc.sync.dma_start(
                wqf[:], d_wq[_NL - 1].rearrange("(c p) n -> p c n", p=_P)
            )
            fcorr = (_NL - 1) in wc_idx
            if fcorr:
                dwf = dwp.tile([_P, _KC, _D], f8, tag="dw")
                nc.sync.dma_start(
                    dwf[:],
                    d_dw[wc_idx[_NL - 1]].rearrange("(c p) n -> p c n", p=_P),
                )
            for bt in range(_BT):
                h, btq = bt // 4, bt % 4
                avr, dv8, v8 = avrh[h], dv8h[h], v8h[h]
                bsl = slice(btq * _P, (btq + 1) * _P)
                for j0 in (0, _BCH):
                    nsl = slice(j0, j0 + _BCH)
                    pm = psf.tile([_P, _BCH], f32, tag="fm")
                    for c in range(_KC):
                        nc.tensor.matmul(
                            pm[:], avr[:, c, bsl], wrf[:, c, nsl],
                            start=(c == 0), stop=False,
                        )
                    ncorr = _KC // 2
                    for c2 in range(ncorr):
                        psl = slice(2 * c2, 2 * c2 + 2)
                        nc.tensor.matmul(
                            pm[:], dv8[:, psl, bsl], wqf[:, psl, nsl],
                            start=False,
                            stop=(not fcorr and c2 == ncorr - 1),
                            perf_mode=DR,
                        )
                    if fcorr:
                        for c2 in range(ncorr):
                            psl = slice(2 * c2, 2 * c2 + 2)
                            nc.tensor.matmul(
                                pm[:], v8[:, psl, bsl], dwf[:, psl, nsl],
                                start=False, stop=(c2 == ncorr - 1),
                                perf_mode=DR,
                            )
                    # sign is invariant to the positive 2^16 scale, so both
                    # outputs read PSUM directly; no separate combine needed.
                    # sign on DVE: (pm >= 0) - 0.5 -> {-0.5, +0.5} in fp8;
                    # the host maps back to +-1
                    sg = tmp.tile([_P, _BCH], f8, tag="fsg", bufs=2)
                    nc.vector.scalar_tensor_tensor(
                        out=sg[:], in0=pm[:], scalar=0.0, in1=half[:],
                        op0=ALU.is_ge, op1=ALU.subtract,
                    )
                    ab = tmp.tile([_P, _BCH], f32, tag="u", bufs=2)
                    nc.scalar.activation(ab[:], pm[:], AF.Abs, scale=2.0 ** -16)
                    lg = tmp.tile([_P, _BCH], f16, tag="lg", bufs=2)
                    nc.scalar.activation(lg[:], ab[:], AF.Ln)
                    # outputs split across the ACT and Pool DMA queues
                    # (both near-idle at this stage), off the weight queue
                    gsl = slice(bt * _P, (bt + 1) * _P)
                    nc.scalar.dma_start(d_sgo[gsl, nsl], sg[:])
                    nc.sync.dma_start(d_lgo[gsl, nsl], lg[:])
    nc.compile()
    return nc


def _rb11(x):
    """Round to 11 explicit mantissa bits (round-to-nearest-even), matching
    the TRN2 fp32r operand rounding bit-for-bit."""
    x = np.asarray(x, dtype=np.float64)
    with np.errstate(divide="ignore", invalid="ignore"):
        e = np.floor(np.log2(np.abs(x)))
    e = np.where(np.isfinite(e), e, 0.0)
    s = 2.0 ** (e - 11)
    return np.round(x / s) * s


def kernel(sign_x, log_abs_x, inner_kernels, final_kernel):
    global _cached_nc, last_results
    import ml_dtypes
    from concourse.bass_utils import run_bass_kernel_spmd

    if _cached_nc is None:
        _cached_nc = _build()
    nc = _cached_nc

    f8 = ml_dtypes.float8_e4m3
    # strip/half-major packing [chunk-of-batch, partition, chunk, col] so
    # each DMA reads contiguous >=512B runs per partition
    def _pack(x, w):  # x: [B, D] -> [B//w, P, KC, w]
        xt = np.asarray(x).T  # [D, B]
        return np.ascontiguousarray(
            xt.reshape(_KC, _P, _B // w, w).transpose(2, 1, 0, 3)
        )

    sign_p = _pack(np.asarray(sign_x, dtype=np.float32), _BCH).astype(f8)
    lab_p = _pack(np.asarray(log_abs_x, dtype=np.float32), _SW)
    ns_core = _NS  # strips per core
    ik = np.asarray(inner_kernels, dtype=np.float32)
    fk = np.asarray(final_kernel, dtype=np.float32)

    W = np.concatenate([ik[:, :_D, :], fk[None]], axis=0)  # [8, 1024, 1024]
    # One PSUM accumulation group at scale 2^16: the fp32r main weights are
    # pre-scaled by 2^16 (exact power of two, commutes with the PE's 11-bit
    # rounding); the corrections' operand scales multiply out to 2^16 too
    # (dv8 at 4096 x wq8 at 16; v8 at 1 x dw8 at 65536). tanh/abs read PSUM
    # with scale=2^-16.
    Wr = np.ascontiguousarray((W.astype(np.float64) * 65536.0).astype(np.float32))
    Wq = (W.astype(np.float64) * 16.0).astype(f8)
    dW = ((W.astype(np.float64) - _rb11(W)) * 65536.0).astype(f8)
    dW = np.ascontiguousarray(dW[list(_WCORR)] if _WCORR else dW[:1] * 0)
    bias = np.ascontiguousarray(
        ik[:, _D, :].reshape(_NL - 1, _NT, _P).transpose(2, 0, 1).reshape(_P, -1)
    )  # [128, 56]: column (l*8+t) holds bias[l, t*128+p] on partition p

    in_maps = []
    for cid in range(_NCORES):
        sl = slice(cid * ns_core, (cid + 1) * ns_core)
        in_maps.append({
            "sign_xt": np.ascontiguousarray(sign_p[2 * cid : 2 * cid + 2]),
            "log_abs_xt": np.ascontiguousarray(lab_p[sl]),
            "wr": Wr,
            "wq": np.ascontiguousarray(Wq),
            "dw": dW,
            "bias": bias,
        })

    last_results = run_bass_kernel_spmd(nc, in_maps, core_ids=list(range(_NCORES)))
    sg = np.sign(np.concatenate(
        [r["out_sg"].astype(np.float32) for r in last_results.results], axis=0
    ))
    lg = np.concatenate(
        [r["out_lg"].astype(np.float32) for r in last_results.results], axis=0
    )
    return np.stack([sg, lg], axis=0)
